# revision 18
# baseline (speedup 1.0000x reference)
"""Trainium2 Bass kernel for nn_EncoderBlock (dual self-attention + BN + FFN + BN).

Sharding: data-parallel over batch (16 batches -> 2 per core on 8 cores).
Device layout: activations transposed (channels E on partitions, tokens on free
dim) so BatchNorm stats are free-dim reductions. Attention computes transposed
scores sT[h] = k_h @ q_h.T so softmax needs no on-device transposes; a ones
column appended to V produces softmax denominators inside the AV matmul; the
per-query reciprocal denominators are broadcast across partitions with a tiny
K=2 matmul. All big matmuls run in float32r (full PE rate, ~1.5e-4 rel err).
BatchNorm batch stats use a 4KB AllReduce across the 8 cores (twice).
The attention phases are software-pipelined at emission time so the per-pair
softmax-denominator chains and batch transitions hide behind independent
projection matmuls.
"""

import numpy as np
import concourse.bass as bass
import concourse.bacc as bacc
import concourse.tile as tile
from concourse import mybir
from concourse.bass_utils import run_bass_kernel_spmd

dt = mybir.dt
F32 = dt.float32
F32R = dt.float32r
AF = mybir.ActivationFunctionType
OP = mybir.AluOpType

N_CORES = 8
B, N, E, H, DK = 16, 1024, 512, 8, 64
NR, NT = 256, 768          # robot / task sequence lengths
BL = B // N_CORES          # local batches per core
TOK = BL * N               # local tokens per core
EC = E // 128              # channel chunks of 128
N_GLOBAL = B * N           # BN stat count
EPS = 1e-5

W_NAMES = ["rq", "rk", "rv", "ro", "tq", "tk", "tv", "to", "f1", "f2"]
ALL_B = W_NAMES + ["bn1_g", "bn1_b", "bn2_g", "bn2_b"]


def _bank_slices(base, length):
    """Split [base, base+length) into pieces (<=512) that never cross a
    512-col PSUM bank boundary. base/length multiples of 256."""
    out = []
    cur = base
    end = base + length
    while cur < end:
        nb = (cur // 512 + 1) * 512
        fl = min(end, nb) - cur
        out.append((cur - base, fl))
        cur += fl
    return out


def build(for_timing=False):
    nc = bacc.Bacc("TRN2", target_bir_lowering=False, debug=False,
                   num_devices=N_CORES)

    xT_d = nc.dram_tensor("xT", [E, TOK], F32, kind="ExternalInput")
    w_d = {n: nc.dram_tensor(f"{n}_wT", [E, E], F32, kind="ExternalInput")
           for n in W_NAMES}
    bpk_d = nc.dram_tensor("bpk", [128, 14 * EC], F32, kind="ExternalInput")
    vrep_d = {n: nc.dram_tensor(f"{n}_brep", [128, E], F32, kind="ExternalInput")
              for n in ["rv", "tv"]}
    sel2_d = nc.dram_tensor("sel2", [2, 128], F32, kind="ExternalInput")
    ones_d = nc.dram_tensor("onesv", [128, H], F32, kind="ExternalInput")
    yT_d = nc.dram_tensor("yT", [E, TOK], F32, kind="ExternalOutput")

    from contextlib import ExitStack
    with tile.TileContext(nc) as tc, ExitStack() as es:
        const = es.enter_context(tc.tile_pool(name="const", bufs=1))
        wpool = es.enter_context(tc.tile_pool(name="w", bufs=1))
        act = es.enter_context(tc.tile_pool(name="act", bufs=1))
        attn = es.enter_context(tc.tile_pool(name="attn", bufs=1))
        expp = es.enter_context(tc.tile_pool(name="expp", bufs=2))
        small = es.enter_context(tc.tile_pool(name="small", bufs=2))
        dram = es.enter_context(tc.tile_pool(name="dram", bufs=1, space="DRAM"))
        ps_big = es.enter_context(tc.tile_pool(name="ps_big", bufs=2, space="PSUM"))
        ps_av = es.enter_context(tc.tile_pool(name="ps_av", bufs=2, space="PSUM"))
        _body(nc, const, wpool, act, attn, expp, small, dram, ps_big, ps_av,
              xT_d, w_d, bpk_d, vrep_d, sel2_d, ones_d, yT_d, for_timing)
    nc.finalize()
    return nc


def _load_w(nc, wpool, w_dram, tag):
    tiles = []
    for k in range(EC):
        t = wpool.tile([128, E], F32R, tag=f"{tag}{k}", name=f"{tag}{k}")
        nc.sync.dma_start(out=t[:],
                          in_=w_dram.ap()[k * 128:(k + 1) * 128, :].bitcast(F32R))
        tiles.append(t)
    return tiles


class _Ctx:
    pass


def _body(nc, const, wpool, act, attn, expp, small, dram, ps_big, ps_av,
          xT_d, w_d, bpk_d, vrep_d, sel2_d, ones_d, yT_d, for_timing):
    # ---------- constants / inputs resident in SBUF ----------
    # DMA emission order == HWDGE issue order: the first projection needs
    # bpk + wq + xT robot-b0 columns; everything else can trickle in after
    bpk = const.tile([128, 14 * EC], F32, tag="bpk", name="bpk")
    nc.sync.dma_start(out=bpk[:], in_=bpk_d.ap())
    bias = {n: bpk[:, i * EC:(i + 1) * EC] for i, n in enumerate(ALL_B)}

    w_robot = {"rq": _load_w(nc, wpool, w_d["rq"], "wq")}
    xT = [const.tile([128, TOK], F32R, tag=f"xT{k}", name=f"xT{k}")
          for k in range(EC)]
    for k in range(EC):
        nc.sync.dma_start(out=xT[k][:, 0:NR],
                          in_=xT_d.ap()[k * 128:(k + 1) * 128, 0:NR].bitcast(F32R))
    w_robot["rk"] = _load_w(nc, wpool, w_d["rk"], "wk")
    w_robot["rv"] = _load_w(nc, wpool, w_d["rv"], "wv")
    vrep = {}
    for n in ["rv", "tv"]:
        t = const.tile([128, E], F32, tag=f"vr_{n}", name=f"vr_{n}")
        nc.sync.dma_start(out=t[:], in_=vrep_d[n].ap())
        vrep[n] = t
    w_robot["ro"] = _load_w(nc, wpool, w_d["ro"], "wo")
    sel2 = const.tile([2, 128], F32R, tag="sel2", name="sel2")
    nc.sync.dma_start(out=sel2[:], in_=sel2_d.ap().bitcast(F32R))
    for off, ln in [(N, NR), (NR, NT), (N + NR, NT)]:
        for k in range(EC):
            nc.sync.dma_start(
                out=xT[k][:, off:off + ln],
                in_=xT_d.ap()[k * 128:(k + 1) * 128, off:off + ln].bitcast(F32R))

    # prefetch the exp ACT table set while input DMAs are in flight
    warm = const.tile([1, 1], F32, tag="warm", name="warm")
    nc.vector.memset(warm[:], 0.0)
    nc.scalar.activation(out=warm[:], in_=warm[:], func=AF.Exp, scale=1.0)

    # persistent V tiles ([128, H, DK+1] per 128-token chunk); the ones
    # column (softmax denominator trick) is initialized once
    v_sb = []
    for t in range(NT // 128):
        vt = attn.tile([128, H, DK + 1], F32R, tag=f"v{t}", name=f"v{t}")
        nc.gpsimd.dma_start(out=vt[:, :, DK:DK + 1],
                            in_=ones_d.ap().bitcast(F32R))
        v_sb.append(vt)

    # h-tilde (pre-BN1 attention output) accumulated across parts/batches
    ht = [act.tile([128, TOK], F32, tag=f"ht{k}", name=f"ht{k}")
          for k in range(EC)]

    # ---------- attention (emission software-pipelined) ----------
    def proj_qkv(P, b):
        """q/k projections into channel-major tiles + v into token-major."""
        tok0 = b * N + (0 if P.part == 0 else NR)
        st = _Ctx()
        st.tok0 = tok0
        st.qT = [attn.tile([128, NT], F32R, tag=f"qT{m}", name=f"qT{m}")
                 for m in range(EC)]
        st.kT = [attn.tile([128, NT], F32R, tag=f"kT{m}", name=f"kT{m}")
                 for m in range(EC)]
        for w_t, o_t, bn_ in ((P.wq, st.qT, P.wn[0]), (P.wk, st.kT, P.wn[1])):
            for m in range(EC):
                ps = ps_big.tile([128, NT], F32, tag="sc", name="psq")
                for off, fl in _bank_slices(0, P.np):
                    for k in range(EC):
                        nc.tensor.matmul(
                            ps[:, off:off + fl],
                            w_t[k][:, m * 128:(m + 1) * 128],
                            xT[k][:, tok0 + off:tok0 + off + fl],
                            start=(k == 0), stop=(k == EC - 1))
                nc.vector.tensor_scalar(
                    out=o_t[m][:, 0:P.np], in0=ps[:, 0:P.np],
                    scalar1=bias[bn_][:, m:m + 1], scalar2=None, op0=OP.add)
        for t in range(P.nk):
            vt = v_sb[t]
            ps = ps_big.tile([128, E], F32, tag="sc", name="psv")
            for k in range(EC):
                nc.tensor.matmul(
                    ps[:], xT[k][:, tok0 + t * 128:tok0 + (t + 1) * 128],
                    P.wv[k][:], start=(k == 0), stop=(k == EC - 1))
            nc.vector.tensor_tensor(
                out=vt[:, :, 0:DK],
                in0=ps[:].rearrange("p (h d) -> p h d", h=H),
                in1=vrep[P.wn[2]][:].rearrange("p (h d) -> p h d", h=H),
                op=OP.add)
        return st

    def heads(P, st):
        """Per-head scores -> exp -> AV (+denominator row); psum evacuated
        partition-aligned then remapped into pair tiles via SWDGE DMA."""
        Np = P.np
        st.zT = [attn.tile([128, NT], F32R, tag=f"zT{p}", name=f"zT{p}")
                 for p in range(4)]
        st.rows = []
        for pair in range(4):
            rows = small.tile([2, NT], F32, tag="rows", name="rows", bufs=2)
            st.rows.append(rows)
            for j in range(2):
                h = 2 * pair + j
                qh = st.qT[h // 2][(h % 2) * 64:(h % 2) * 64 + 64, 0:Np]
                kh = st.kT[h // 2][(h % 2) * 64:(h % 2) * 64 + 64, 0:Np]
                zu = ps_av.tile([65, NT], F32, tag="av", name="av")
                if P.part == 0:
                    sc = ps_big.tile([128, 512], F32, tag="sc", name="sc")
                    for kc in range(P.nk):
                        nc.tensor.matmul(sc[:, kc * Np:(kc + 1) * Np],
                                         kh[:, kc * 128:(kc + 1) * 128], qh,
                                         start=True, stop=True)
                    ex = expp.tile([128, 512], F32R, tag="exp", name="exp")
                    nc.scalar.activation(out=ex[:], in_=sc[:], func=AF.Exp,
                                         scale=0.125)
                    for kc in range(P.nk):
                        for off, fl in _bank_slices(0, Np):
                            nc.tensor.matmul(
                                zu[:, off:off + fl], v_sb[kc][:, h, :],
                                ex[:, kc * Np + off:kc * Np + off + fl],
                                start=(kc == 0), stop=(kc == P.nk - 1))
                else:
                    for kc in range(P.nk):
                        sc = ps_big.tile([128, NT], F32, tag="sc", name="sc")
                        for off, fl in _bank_slices(0, Np):
                            nc.tensor.matmul(sc[:, off:off + fl],
                                             kh[:, kc * 128:(kc + 1) * 128],
                                             qh[:, off:off + fl],
                                             start=True, stop=True)
                        ex = expp.tile([128, NT], F32R, tag="exp", name="exp")
                        nc.scalar.activation(out=ex[:], in_=sc[:], func=AF.Exp,
                                             scale=0.125)
                        for off, fl in _bank_slices(0, Np):
                            nc.tensor.matmul(
                                zu[:, off:off + fl], v_sb[kc][:, h, :],
                                ex[:, off:off + fl],
                                start=(kc == 0), stop=(kc == P.nk - 1))
                zst = expp.tile([65, NT], F32R, tag="zst", name="zst", bufs=2)
                nc.vector.tensor_copy(out=zst[:, 0:Np],
                                      in_=zu[:, 0:Np].bitcast(F32R))
                nc.sync.dma_start(out=st.zT[pair][j * 64:(j + 1) * 64, 0:Np],
                                  in_=zst[0:64, 0:Np])
                nc.sync.dma_start(out=rows[j:j + 1, 0:Np],
                                  in_=zst[64:65, 0:Np].bitcast(F32))

    def denom_outproj(P, st):
        """Softmax denominators (reciprocal + K=2 broadcast matmul), then the
        output projection with bias + residual into ht."""
        Np = P.np
        tok0 = st.tok0
        for pair in range(4):
            rinv = small.tile([2, NT], F32R, tag="rinv", name="rinv", bufs=1)
            with nc.allow_low_precision(reason="f32r feeds f32r matmul"):
                nc.vector.reciprocal(out=rinv[:, 0:Np],
                                     in_=st.rows[pair][:, 0:Np])
            rep = ps_av.tile([128, NT], F32, tag="av", name="rep")
            for off, fl in _bank_slices(0, Np):
                nc.tensor.matmul(rep[:, off:off + fl], sel2[:],
                                 rinv[:, off:off + fl], start=True, stop=True)
            nc.vector.tensor_tensor(out=st.zT[pair][:, 0:Np],
                                    in0=st.zT[pair][:, 0:Np].bitcast(F32),
                                    in1=rep[:, 0:Np], op=OP.mult)
        for m in range(EC):
            ps = ps_big.tile([128, NT], F32, tag="sc", name="pso")
            for off, fl in _bank_slices(0, Np):
                for k in range(EC):
                    nc.tensor.matmul(ps[:, off:off + fl],
                                     P.wo[k][:, m * 128:(m + 1) * 128],
                                     st.zT[k][:, off:off + fl],
                                     start=(k == 0), stop=(k == EC - 1))
            dst = ht[m][:, tok0:tok0 + Np]
            nc.scalar.activation(out=dst, in_=ps[:, 0:Np], func=AF.Identity,
                                 bias=bias[P.wn[3]][:, m:m + 1], scale=1.0)
            nc.vector.tensor_tensor(
                out=dst, in0=dst,
                in1=xT[m][:, tok0:tok0 + Np].bitcast(F32), op=OP.add)
            if P.part == 1 and tok0 >= N:
                # batch-1 token chunks complete for this m: emit BN1 stats
                for c in (2, 3):
                    nc.vector.bn_stats(out=st1_tiles[m][:, c, :],
                                       in_=ht[m][:, c * 512:(c + 1) * 512])

    st1_tiles = _bn_stats_tiles(small, "bn1")
    st2_tiles = _bn_stats_tiles(small, "bn2")
    f1 = f2 = None
    for part in range(2):
        P = _Ctx()
        P.part = part
        P.wn = ["rq", "rk", "rv", "ro"] if part == 0 else ["tq", "tk", "tv", "to"]
        P.np = NR if part == 0 else NT
        P.nk = P.np // 128
        if part == 0:
            P.wq, P.wk, P.wv, P.wo = (w_robot["rq"], w_robot["rk"],
                                      w_robot["rv"], w_robot["ro"])
        else:
            P.wq = _load_w(nc, wpool, w_d[P.wn[0]], "wq")
            P.wk = _load_w(nc, wpool, w_d[P.wn[1]], "wk")
            P.wv = _load_w(nc, wpool, w_d[P.wn[2]], "wv")
            P.wo = _load_w(nc, wpool, w_d[P.wn[3]], "wo")

        st0 = proj_qkv(P, 0)
        heads(P, st0)
        st1 = proj_qkv(P, 1)
        denom_outproj(P, st0)
        if part == 1:
            # ht token chunks 0,1 (batch 0) are complete: emit their BN1 stats
            for m in range(EC):
                for c in (0, 1):
                    nc.vector.bn_stats(
                        out=st1_tiles[m][:, c, :],
                        in_=ht[m][:, c * 512:(c + 1) * 512])
        heads(P, st1)
        if part == 1:
            # prefetch FFN weights into slots whose last readers are done
            f1 = _load_w(nc, wpool, w_d["f1"], "wq")
            f2 = _load_w(nc, wpool, w_d["f2"], "wk")
            # all exps done: swap the ACT table set to sqrt ahead of BN1
            warm2 = const.tile([1, 1], F32, tag="warm", name="warm2")
            nc.vector.memset(warm2[:], 1.0)
            nc.scalar.activation(out=warm2[:], in_=warm2[:], func=AF.Sqrt,
                                 scale=1.0)
        denom_outproj(P, st1)

    # ---------- BN1 ----------
    s1, t1 = _bn_params(nc, small, dram, st1_tiles, bias["bn1_g"],
                        bias["bn1_b"], "bn1", for_timing)
    hn = [act.tile([128, TOK], F32R, tag=f"hn{k}", name=f"hn{k}")
          for k in range(EC)]
    for m in range(EC):
        if m % 2 == 0:
            nc.vector.tensor_scalar(out=hn[m][:], in0=ht[m][:],
                                    scalar1=s1[m], scalar2=t1[m],
                                    op0=OP.mult, op1=OP.add)
        else:
            nc.scalar.activation(out=hn[m][:], in_=ht[m][:], func=AF.Identity,
                                 bias=t1[m], scale=s1[m])

    # ---------- FFN ----------
    h1 = [const.tile([128, TOK], F32R, tag=f"xT{k}", name=f"h1_{k}")
          for k in range(EC)]
    for m in range(EC):
        for off, fl in _bank_slices(0, TOK):
            ps = ps_big.tile([128, 512], F32, tag="sc", name="psf1")
            for k in range(EC):
                nc.tensor.matmul(ps[:, 0:fl], f1[k][:, m * 128:(m + 1) * 128],
                                 hn[k][:, off:off + fl],
                                 start=(k == 0), stop=(k == EC - 1))
            nc.scalar.activation(out=h1[m][:, off:off + fl], in_=ps[:, 0:fl],
                                 func=AF.Relu, bias=bias["f1"][:, m:m + 1],
                                 scale=1.0)
    ho = [act.tile([128, TOK], F32, tag=f"ht{k}", name=f"ho{k}")
          for k in range(EC)]
    for m in range(EC):
        for off, fl in _bank_slices(0, TOK):
            ps = ps_big.tile([128, 512], F32, tag="sc", name="psf2")
            for k in range(EC):
                nc.tensor.matmul(ps[:, 0:fl], f2[k][:, m * 128:(m + 1) * 128],
                                 h1[k][:, off:off + fl],
                                 start=(k == 0), stop=(k == EC - 1))
            dst = ho[m][:, off:off + fl]
            nc.scalar.activation(out=dst, in_=ps[:, 0:fl], func=AF.Identity,
                                 bias=bias["f2"][:, m:m + 1], scale=1.0)
            nc.vector.tensor_tensor(out=dst, in0=dst,
                                    in1=hn[m][:, off:off + fl].bitcast(F32),
                                    op=OP.add)
            nc.vector.bn_stats(out=st2_tiles[m][:, off // 512, :], in_=dst)

    # ---------- BN2 + output (pipelined per 512-token slice) ----------
    s2, t2 = _bn_params(nc, small, dram, st2_tiles, bias["bn2_g"],
                        bias["bn2_b"], "bn2", for_timing)
    for m in range(EC):
        if m % 2 == 0:
            nc.vector.tensor_scalar(out=ho[m][:], in0=ho[m][:],
                                    scalar1=s2[m], scalar2=t2[m],
                                    op0=OP.mult, op1=OP.add)
        else:
            nc.scalar.activation(out=ho[m][:], in_=ho[m][:],
                                 func=AF.Identity, bias=t2[m], scale=s2[m])
        nc.sync.dma_start(out=yT_d.ap()[m * 128:(m + 1) * 128, :], in_=ho[m][:])


def _bn_stats_tiles(small, name):
    return [small.tile([128, 4, 6], F32, tag=f"st_{name}{m}",
                       name=f"st_{name}{m}", bufs=1) for m in range(EC)]


def _bn_params(nc, small, dram, sts, g_sb, b_sb, name, for_timing=False):
    """Per-channel scale/shift for training-mode BN over all B*N tokens:
    local sums (bn_stats emitted earlier into sts) -> 8-core AllReduce ->
    mu/var -> sqrt+recip (+1 Newton step).
    Returns ([EC] scale APs, [EC] shift APs), each [128, 1]."""
    ccin = dram.tile([128, 2 * EC], F32, tag=f"cci_{name}", name=f"cci_{name}")
    ccout = dram.tile([128, 2 * EC], F32, tag=f"cco_{name}", name=f"cco_{name}")
    su = small.tile([128, 2 * EC], F32, tag=f"su_{name}", name=f"su_{name}")
    mva = small.tile([128, EC, 2], F32, tag=f"mv_{name}", name=f"mv_{name}",
                     bufs=1)
    for m in range(EC):
        nc.vector.bn_aggr(out=mva[:, m, :], in_=sts[m][:])
    # su0 = sum(h) = mean * TOK ; su1 = sum(h^2) = (var + mean^2) * TOK
    suv = su[:].rearrange("p (c two) -> p c two", two=2)
    t = small.tile([128, EC], F32, tag=f"tmp_{name}", name=f"tmp_{name}",
                   bufs=1)
    nc.vector.tensor_scalar(out=suv[:, :, 0], in0=mva[:, :, 0],
                            scalar1=float(TOK), scalar2=None, op0=OP.mult)
    nc.vector.tensor_tensor(out=t[:], in0=mva[:, :, 0], in1=mva[:, :, 0],
                            op=OP.mult)
    nc.vector.tensor_tensor(out=t[:], in0=t[:], in1=mva[:, :, 1], op=OP.add)
    nc.vector.tensor_scalar(out=suv[:, :, 1], in0=t[:], scalar1=float(TOK),
                            scalar2=None, op0=OP.mult)
    nc.sync.dma_start(out=ccin[:], in_=su[:])
    if for_timing:
        # TimelineSim cannot model collectives; substitute a same-shape copy
        nc.gpsimd.dma_start(out=ccout[:], in_=ccin[:])
    else:
        nc.gpsimd.collective_compute(
            "AllReduce", OP.add, replica_groups=[list(range(N_CORES))],
            ins=[ccin.opt()], outs=[ccout.opt()])
    scales, shifts = [], []
    gsa = small.tile([128, 2 * EC], F32, tag=f"gs_{name}", name=f"gs_{name}")
    nc.sync.dma_start(out=gsa[:], in_=ccout[:])
    gv = gsa[:].rearrange("p (c two) -> p c two", two=2)
    mu = small.tile([128, EC], F32, tag=f"mu_{name}", name=f"mu_{name}", bufs=1)
    var = small.tile([128, EC], F32, tag=f"var_{name}", name=f"var_{name}",
                     bufs=1)
    t2 = small.tile([128, EC], F32, tag=f"t2_{name}", name=f"t2_{name}", bufs=1)
    nc.vector.tensor_scalar(out=mu[:], in0=gv[:, :, 0],
                            scalar1=1.0 / N_GLOBAL, scalar2=None, op0=OP.mult)
    nc.vector.tensor_scalar(out=t2[:], in0=gv[:, :, 1],
                            scalar1=1.0 / N_GLOBAL, scalar2=None, op0=OP.mult)
    nc.vector.tensor_tensor(out=var[:], in0=mu[:], in1=mu[:], op=OP.mult)
    nc.vector.tensor_tensor(out=var[:], in0=t2[:], in1=var[:], op=OP.subtract)
    # r = 1/sqrt(var + eps): ACT Sqrt + DVE reciprocal, then one Newton step
    # to wash out the sqrt table's loose ULP budget
    epst = small.tile([128, 1], F32, tag=f"eps_{name}", name=f"eps_{name}",
                      bufs=1)
    nc.vector.memset(epst[:], EPS)
    sq = small.tile([128, EC], F32, tag=f"sq_{name}", name=f"sq_{name}", bufs=1)
    nc.scalar.activation(out=sq[:], in_=var[:], func=AF.Sqrt, bias=epst[:],
                         scale=1.0)
    r0 = small.tile([128, EC], F32, tag=f"r0_{name}", name=f"r0_{name}", bufs=1)
    nc.vector.reciprocal(out=r0[:], in_=sq[:])
    av_ = small.tile([128, EC], F32, tag=f"a_{name}", name=f"a_{name}", bufs=1)
    nc.vector.tensor_scalar(out=av_[:], in0=var[:], scalar1=EPS, scalar2=None,
                            op0=OP.add)
    nt = small.tile([128, EC], F32, tag=f"nt_{name}", name=f"nt_{name}", bufs=1)
    nc.vector.tensor_tensor(out=nt[:], in0=r0[:], in1=r0[:], op=OP.mult)
    nc.vector.tensor_tensor(out=nt[:], in0=nt[:], in1=av_[:], op=OP.mult)
    nc.vector.tensor_scalar(out=nt[:], in0=nt[:], scalar1=-0.5, scalar2=1.5,
                            op0=OP.mult, op1=OP.add)
    r = small.tile([128, EC], F32, tag=f"r_{name}", name=f"r_{name}", bufs=1)
    nc.vector.tensor_tensor(out=r[:], in0=r0[:], in1=nt[:], op=OP.mult)
    s_all = small.tile([128, EC], F32, tag=f"s_{name}", name=f"s_{name}",
                       bufs=1)
    sh_all = small.tile([128, EC], F32, tag=f"sh_{name}", name=f"sh_{name}",
                        bufs=1)
    nc.vector.tensor_tensor(out=s_all[:], in0=r[:], in1=g_sb, op=OP.mult)
    nc.vector.tensor_tensor(out=sh_all[:], in0=mu[:], in1=s_all[:], op=OP.mult)
    nc.vector.tensor_tensor(out=sh_all[:], in0=b_sb, in1=sh_all[:],
                            op=OP.subtract)
    for m in range(EC):
        scales.append(s_all[:, m:m + 1])
        shifts.append(sh_all[:, m:m + 1])
    return scales, shifts


_NC_CACHE = None


def _get_nc():
    global _NC_CACHE
    if _NC_CACHE is None:
        _NC_CACHE = build()
    return _NC_CACHE


def make_in_maps(inputs):
    shared = {}
    for n in W_NAMES:
        shared[f"{n}_wT"] = np.ascontiguousarray(inputs[f"{n}_w"].T)
    for n in ["rv", "tv"]:
        shared[f"{n}_brep"] = np.ascontiguousarray(
            np.broadcast_to(inputs[f"{n}_b"], (128, E)))
    bpk = np.empty((128, 14 * EC), dtype=np.float32)
    for i, n in enumerate(ALL_B):
        vec = inputs[f"{n}_b"] if n in W_NAMES else inputs[n]
        bpk[:, i * EC:(i + 1) * EC] = np.asarray(vec).reshape(EC, 128).T
    shared["bpk"] = bpk
    sel2 = np.zeros((2, 128), dtype=np.float32)
    sel2[0, 0:64] = 1.0
    sel2[1, 64:128] = 1.0
    shared["sel2"] = sel2
    shared["onesv"] = np.ones((128, H), dtype=np.float32)

    x = np.asarray(inputs["x"], dtype=np.float32)
    in_maps = []
    for i in range(N_CORES):
        xc = x[BL * i:BL * (i + 1)]                      # [BL, N, E]
        xT = np.ascontiguousarray(xc.transpose(2, 0, 1).reshape(E, TOK))
        in_maps.append({"xT": xT, **shared})
    return in_maps


def assemble_output(results):
    y = np.empty((B, N, E), dtype=np.float32)
    for i in range(N_CORES):
        yT = results[i]["yT"]                            # [E, TOK]
        y[BL * i:BL * (i + 1)] = yT.reshape(E, BL, N).transpose(1, 2, 0)
    return y


def kernel(**inputs):
    nc = _get_nc()
    in_maps = make_in_maps(inputs)
    res = run_bass_kernel_spmd(nc, in_maps, core_ids=list(range(N_CORES)))
    return assemble_output(res.results)


if __name__ == "__main__":
    nc = build()
    print("build ok")
